# revision 46
# baseline (speedup 1.0000x reference)
"""Trainium2 Bass kernel for the LSTM GAN-discriminator problem.

Math (reference): two 16-step LSTM passes over [B=4096, T=16, F=64] sharing the
first PREV=6 steps (fake sequence = real[:, :6] ++ fake_input), then a dense+
sigmoid head on hidden states of steps 6..15 of each pass.

Strategy:
  - Data parallel: batch 4096 -> 8 cores x 512 rows; weights replicated.
  - Shared prefix: 6 cells at N=512, then the two branches run as separate
    interleaved chains (10 real + 10 fake cells), 26 cell evaluations total.
  - Transposed layout: features on partitions, batch on the free dim. The
    4H=1024 gate columns live as 8 "banks" of 128 partitions; hidden state
    h lives as [128, 2, N] (2 k-tiles of 128 features), so the recurrent
    matmul contracts K=128 per k-tile with NO per-step transpose.
  - Bias is folded into the x-projection via an augmented ones-row (K=65).
  - All matmul operands bf16 (PSUM accumulates fp32); gates/c/h bf16 for
    the 2x DVE tensor_tensor mode. (fp8 DoubleRow for the recurrent matmul
    was tried twice: the non-FWL 256-column LDWEIGHTS erases the matmul
    saving and the 1x-rate fp8-out DVE h-op lengthens the recurrence
    chain -- HW-slower both times. bf16 kept.)
  - ACT is the co-bottleneck with PE (~5.1us/cell each), so ACT work is
    minimized structurally:
      * g-gate tanh becomes sigmoid: the g columns of wx/wh/bias are
        pre-doubled on the host (tanh(z) = 2*sigmoid(2z) - 1), so PSUM
        splits as [i,i,f,f] / [o,o,g,g] (4+4 banks, bufs=1) and ONE
        2048-col sigmoid drains each tile -- 2 gate ACT ops per cell
        instead of 3, and a tile's drain latency stays under the PE
        period (a 6+2 split with a 3072-col sigmoid stalls the next
        cell's matmuls on the drain).
      * The c-update absorbs the g rescale off the serial chain:
        c = f*c_prev + i*(2*sg-1) is computed as (f*c_prev - i) + (2i)*sg,
        so only two DVE ops sit between sigmoid([o,g]) and tanh(c).
      * Dense head: steps packed 4-at-a-time (4+4+2) into a 4-bank PSUM
        tile -- M=1 matmuls col-packed via tile_position into partitions
        {0 (real), 32 (fake)}, steps on free quarters -- so THREE
        sigmoids total drain the whole head. (Do not use col-groups
        (0,64)/(0,96): quadrant-3 tile_position wedged the device.)
  - Cell emission is phase-split: phase 1 (matmuls + 2 sigmoids), phase 2
    (DVE combine + tanh(c) + h) is emitted one cell-chunk LATE so tanh(c),
    which waits on the DVE chain, cannot head-of-line-block the next
    cell's already-ready sigmoids in the strict-FIFO ACT queue.
  - A PE warm-up burst (8 matmuls on a zeroed scratch tile) runs during
    the initial weight DMAs so the PE_HAM clock gate reaches 8/8 (2.4 GHz)
    ~10us into the kernel instead of ~23us.
  - Prefix cells run as 2 batch chunks of 256 (pipeline depth for the
    serial chain); branch cells run one chunk of 512, with the two
    branches alternating cells to keep PE/ACT/DVE overlapped.
  - DO NOT reorder a cell's matmuls x-passes-first across banks: the
    interleaved PSUM accumulation groups (start on all 8 banks before any
    stop) produced a 20x accuracy regression on HW. Keep the per-bank
    x, h0, h1 triplets.
  - Measured end-to-end rel err ~4.1e-4 (tolerance 2e-2).
"""

import sys

if "/opt/trn_rl_repo" not in sys.path:
    sys.path.insert(0, "/opt/trn_rl_repo")

import numpy as np
import ml_dtypes

import concourse.mybir as mybir
import concourse.tile as tile
from concourse import bacc

BF16 = ml_dtypes.bfloat16

PREV, PRED, FEAT, HID = 6, 10, 64, 256
B = 4096
N_CORES = 8
BS = B // N_CORES          # 512 rows per core
CH = 256                   # chunk of the per-core batch
NCH = BS // CH             # 2 chunks
NCELL = PREV + 2 * PRED    # 26 cell evaluations per core
H4 = 4 * HID               # 1024

# gate bank order [i_s0, i_s1, f_s0, f_s1, o_s0, o_s1, g_s0, g_s1]
# (original z column order is i, f, g, o)
_GATE_BASE = [0, 0, 256, 256, 768, 768, 512, 512]
PERM = np.concatenate(
    [np.arange(_GATE_BASE[m] + 128 * (m % 2), _GATE_BASE[m] + 128 * (m % 2) + 128)
     for m in range(8)]
)

# canonical cell ids: 0..5 prefix, 6..15 real steps 6..15, 16..25 fake steps 6..15
# processing order interleaves the two independent branches; the fake cell
# goes first in each pair so f6 can read c5 out of c_real before r6
# overwrites it (no c copy needed at the branch point)
CELL_ORDER = list(range(PREV)) + [
    c for t in range(PRED) for c in (PREV + PRED + t, PREV + t)
]


def _h_src(hid_):
    """canonical id of the cell whose h feeds this cell (None for cell 0)."""
    if hid_ == 0:
        return None
    if hid_ == PREV + PRED:  # first fake cell branches off the prefix
        return PREV - 1
    return hid_ - 1


def _build_program(loop_r=None):
    f32 = mybir.dt.float32
    bf16 = mybir.dt.bfloat16
    AF = mybir.ActivationFunctionType
    OP = mybir.AluOpType

    nc = bacc.Bacc("TRN2", target_bir_lowering=False, debug=False,
                   num_devices=N_CORES)

    xT = nc.dram_tensor("xT", [NCELL, FEAT + 1, BS], bf16, kind="ExternalInput").ap()
    wx = nc.dram_tensor("wx", [FEAT + 1, H4], bf16, kind="ExternalInput").ap()
    wh = nc.dram_tensor("wh", [128, 2, H4], bf16, kind="ExternalInput").ap()
    dw = nc.dram_tensor("dw", [128, 2], bf16, kind="ExternalInput").ap()
    dbias = nc.dram_tensor("dbias", [128, 1], mybir.dt.float32,
                           kind="ExternalInput").ap()
    outT = nc.dram_tensor("outT", [2, PRED, BS], f32, kind="ExternalOutput").ap()

    with tile.TileContext(nc) as tc:
        with (
            tc.tile_pool(name="const", bufs=1) as const,
            tc.tile_pool(name="xpool", bufs=8) as xpool,
            tc.tile_pool(name="zpool", bufs=2, space="PSUM") as zpool,
            tc.tile_pool(name="gpool", bufs=3) as gpool,
            tc.tile_pool(name="tpool", bufs=3) as tpool,
        ):
            wx_t = const.tile([FEAT + 1, H4], bf16, tag="wx")
            wh_t = const.tile([128, 2, H4], bf16, tag="wh")
            dw_t = const.tile([128, 2], bf16, tag="dw")
            db_t = const.tile([128, 1], f32, tag="db")
            c_real = const.tile([128, H4], bf16, tag="c_real")
            c_fake = const.tile([128, H4], bf16, tag="c_fake")
            dsig = const.tile([33, PRED * BS], f32, tag="dsig")
            h_tiles = [const.tile([128, 2, BS], bf16, tag=f"h{i}", name=f"h{i}")
                       for i in range(NCELL)]

            # (A PE warm-up matmul burst was tried here to flip the PE_HAM
            # clock gate early; traces showed the flip timestamp unchanged
            # (~22us) -- the prefix phase is latency-bound and stays at
            # K=4/8 regardless -- so it was only a ~3.5us startup delay.)

            # ONE manually-managed PSUM region covering all 8 banks.
            # Branch cells: za = [0:2048] (banks 0-3), zb = [2048:4096].
            # Prefix chunks ping-pong by chunk parity within each half, so
            # both chunks of a cell are in flight with NO serialization on
            # the previous chunk's sigmoid drain (a per-chunk-tile pool at
            # bufs=1 left 4 banks idle and stalled PE ~0.8us per chunk).
            # Tile's range-granular dependency tracking supplies the same
            # WAR/RAW edges the pool rotation did.
            Z = zpool.tile([128, 4096], f32, tag="Z", name="Z", bufs=1)

            # dummy activation: forces the sigmoid/tanh ACT table load to
            # happen during the weight DMAs instead of on the critical path
            warm = tpool.tile([128, 1], f32, tag="warm", name="warm")
            nc.scalar.activation(warm[:, :], db_t[:, :], AF.Sigmoid)
            nc.scalar.activation(warm[:, :], db_t[:, :], AF.Tanh)

            nc.sync.dma_start(wx_t[:, :], wx)
            nc.sync.dma_start(wh_t[:, :, :], wh)
            nc.sync.dma_start(dw_t[:, :], dw)
            nc.sync.dma_start(db_t[:, :], dbias)

            def _nch(hid_):
                return NCH if hid_ < PREV else 1

            gstate = {}

            def emit_p1(hid_, ch, x_t):
                """Phase 1 of one cell-chunk: gate matmuls + the two
                sigmoids. Prefix cells run as 2 batch chunks of 256
                (pipeline depth for the serial chain); branch cells run one
                chunk of 512 (alternating real/fake cells give the pipeline
                parallelism instead). PSUM is split [i,f] / [o,g] (4+4
                banks at N=512, bufs=1 each) so the next cell's matmuls can
                reuse a tile as soon as its sigmoid drains it."""
                h_prev = None if _h_src(hid_) is None else h_tiles[_h_src(hid_)]
                cw = BS // _nch(hid_)

                if _nch(hid_) == 1:
                    za = Z[:, 0:2048]
                    zb = Z[:, 2048:4096]
                else:
                    # prefix: chunk ch owns the contiguous half
                    # Z[ch*2048:(ch+1)*2048] with za|zb adjacent, so ONE
                    # 2048-col sigmoid drains all 8 gate banks of the chunk
                    za = Z[:, ch * 2048:ch * 2048 + 4 * cw]
                    zb = Z[:, ch * 2048 + 4 * cw:(ch + 1) * 2048]
                banks = [(za, m, m) for m in range(4)] + \
                        [(zb, m, m + 4) for m in range(4)]
                for ztile, k, m in banks:
                    zs = ztile[:, k * cw:(k + 1) * cw]
                    nc.tensor.matmul(
                        zs,
                        wx_t[:, m * 128:(m + 1) * 128],
                        x_t[:, ch * cw:(ch + 1) * cw],
                        start=True, stop=(h_prev is None),
                    )
                    if h_prev is not None:
                        for s in range(2):
                            nc.tensor.matmul(
                                zs,
                                wh_t[:, s, m * 128:(m + 1) * 128],
                                h_prev[:, s, ch * cw:(ch + 1) * cw],
                                start=False, stop=(s == 1),
                            )
                # --- ACT: gates (bank order [i i f f] / [o o g g]). The g
                # columns of wx/wh/bias are pre-doubled on the host so
                # tanh(zg) = 2*sigmoid(2*zg) - 1: ONE sigmoid covers
                # [o o g g]; a cheap DVE fixup rescales g. ---
                # ONE sigmoid per PSUM drain region. Branch cells keep the
                # 4+4 split (a single 4096-col sigmoid would serialize the
                # pair cadence on its drain); prefix chunks merge za|zb into
                # one 2048-col sigmoid (2 ACT ops/chunk instead of 3).
                # (Splitting sigmoid(za) into two 1024-col halves to free
                # the banks earlier was A/B-tested: +15us WORSE -- the
                # second op's fixed ~350-cyc overhead sits on the drain
                # path, so the tile frees later.)
                if _nch(hid_) == 1:
                    ga = gpool.tile([128, 4 * cw], bf16, tag="ga", name="ga")
                    gb = gpool.tile([128, 4 * cw], bf16, tag="gb", name="gb")
                    nc.scalar.activation(ga[:, :], za[:, :], AF.Sigmoid)
                    nc.scalar.activation(gb[:, :], zb[:, :], AF.Sigmoid)
                else:
                    gab = gpool.tile([128, 8 * cw], bf16, tag="ga", name="gab")
                    nc.scalar.activation(
                        gab[:, :], Z[:, ch * 2048:(ch + 1) * 2048], AF.Sigmoid)
                    ga = gab[:, 0:4 * cw]
                    gb = gab[:, 4 * cw:8 * cw]
                gstate[(hid_, ch)] = (ga, gb)

            def emit_p2(hid_, ch):
                """Phase 2 of one cell-chunk: DVE gate combine + tanh(c) +
                h. Emitted one cell-chunk BEHIND phase 1 so this chunk's
                tanh(c) (which waits on the DVE chain) enqueues on the ACT
                FIFO after the next chunk's sigmoids -- the strict-FIFO ACT
                queue would otherwise head-of-line block on it."""
                ga, gb = gstate.pop((hid_, ch))
                h_prev = None if _h_src(hid_) is None else h_tiles[_h_src(hid_)]
                c_in = c_out = c_real if hid_ < PREV + PRED else c_fake
                if hid_ == PREV + PRED:
                    c_in = c_real  # branch point: fake chain starts from c5
                nch = _nch(hid_)
                cw = BS // nch

                def v3(t, lo):
                    return t[:, lo:lo + 2 * cw].rearrange(
                        "p (s n) -> p s n", s=2, n=cw)

                i3, f3 = v3(ga, 0), v3(ga, 2 * cw)
                o3, sg3 = v3(gb, 0), v3(gb, 2 * cw)

                def cvw(t):
                    return t[:, :].rearrange("p (s c n) -> p s c n",
                                             s=2, c=nch, n=cw)[:, :, ch]

                cvi, cvo = cvw(c_in), cvw(c_out)
                # c = f*c_prev + i*(2*sg - 1) reassociated as
                #     (f*c_prev - i) + (2i)*sg
                # so everything except the last two DVE ops depends only on
                # the EARLIER sigmoid (za): the serial chain after sigmoid(zb)
                # stays two ops long, same as with a native tanh g-gate.
                i2 = tpool.tile([128, 2 * cw], bf16, tag="g2", name="i2")
                vv = tpool.tile([128, 2 * cw], bf16, tag="vv", name="vv")
                nc.vector.tensor_scalar_mul(v3(i2, 0), i3, 2.0)
                if h_prev is None:
                    nc.vector.tensor_tensor(v3(vv, 0), v3(i2, 0), sg3, OP.mult)
                    nc.vector.tensor_tensor(cvo, v3(vv, 0), i3, OP.subtract)
                else:
                    fc = tpool.tile([128, 2 * cw], bf16, tag="fc", name="fc")
                    ig = tpool.tile([128, 2 * cw], bf16, tag="ig", name="ig")
                    nc.vector.tensor_tensor(v3(fc, 0), f3, cvi, OP.mult)
                    nc.vector.tensor_tensor(v3(ig, 0), v3(fc, 0), i3, OP.subtract)
                    nc.vector.tensor_tensor(v3(vv, 0), v3(i2, 0), sg3, OP.mult)
                    nc.vector.tensor_tensor(cvo, v3(ig, 0), v3(vv, 0), OP.add)
                tcn = tpool.tile([128, 2 * cw], bf16, tag="tc", name="tc")
                nc.scalar.activation(v3(tcn, 0), cvo, AF.Tanh)
                hv = h_tiles[hid_][:, :, ch * cw:(ch + 1) * cw]
                nc.vector.tensor_tensor(hv, o3, v3(tcn, 0), OP.mult)

            def emit_dense(q_):
                """pred[:, 4q:4q+4] for both branches in ONE 4-bank PSUM
                tile: partitions {0 real, 32 fake} via tile_position, the 4
                steps of the quad on free quarters, so a single
                sigmoid(+bias) drains it. 3 emissions total (4+4+2 steps).
                (2-step pairs were A/B-tested: consistently ~4us slower
                single-shot -- the extra sigmoids and extra PSUM-chain
                disruptions outweigh the shorter matmul bursts.)"""
                nstep = min(4, PRED - 4 * q_)
                dp = Z[:, 0:nstep * BS]
                for dt_ in range(nstep):
                    for br, j0 in ((0, 0), (1, 32)):
                        cell = (PREV if br == 0 else PREV + PRED) + 4 * q_ + dt_
                        for s in range(2):
                            nc.tensor.matmul(
                                dp[j0:j0 + 1, dt_ * BS:(dt_ + 1) * BS],
                                dw_t[:, s:s + 1],
                                h_tiles[cell][:, s, :],
                                start=(s == 0), stop=(s == 1),
                                tile_position=(0, j0),
                            )
                nc.scalar.activation(
                    dsig[:, 4 * q_ * BS:(4 * q_ + nstep) * BS],
                    dp[0:33, 0:nstep * BS],
                    AF.Sigmoid, bias=db_t[0:33, 0:1])

            def emit_body():
              units = []
              for hid_ in CELL_ORDER:
                  for ch in range(_nch(hid_)):
                      units.append((hid_, ch))

              x_tiles = {}
              pending = None
              for hid_, ch in units:
                if ch == 0:
                    x_t = xpool.tile([FEAT + 1, BS], bf16, tag="x", name="x")
                    nc.sync.dma_start(x_t[:, :], xT[hid_])
                    x_tiles[hid_] = x_t

                if hid_ == PREV + PRED and pending is not None:
                    # branch point: the first fake cell reads BOTH chunks of
                    # h5, so the delayed phase-2 of (5, 1) must land first
                    emit_p2(*pending)
                    pending = None

                emit_p1(hid_, ch, x_tiles[hid_])
                if pending is not None:
                    emit_p2(*pending)
                pending = (hid_, ch)

                t_r = hid_ - PREV  # real cell completes step t_r
                if PREV <= hid_ < PREV + PRED and t_r in (5, 9):
                    # dense quad (4q..4q+3), emitted behind the scan
                    emit_dense((t_r - 5) // 4)

              emit_p2(*pending)
              emit_dense(2)

              nc.sync.dma_start(outT[0], dsig[0:1, :])
              nc.sync.dma_start(outT[1], dsig[32:33, :])

            if loop_r is None:
                emit_body()
            else:
                with tc.For_i(0, loop_r, 1,
                              hint_engines=(mybir.EngineType.PE,)):
                    emit_body()

    nc.compile()
    return nc


_PROGRAMS = {}


def _get_program(loop_r=None):
    if loop_r not in _PROGRAMS:
        _PROGRAMS[loop_r] = _build_program(loop_r)
    return _PROGRAMS[loop_r]


def _prep_inputs(real_input, fake_input, kernel, recurrent_kernel, bias, dense_w,
                 dense_b):
    kernel_p = np.asarray(kernel, np.float32)[:, PERM]
    bias_p = np.asarray(bias, np.float32)[PERM]
    wh_p = np.asarray(recurrent_kernel, np.float32)[:, PERM]
    # double the g-gate columns (permuted banks 6,7): tanh(z) = 2*sig(2z)-1
    kernel_p[:, 6 * 128:] *= 2.0
    bias_p[6 * 128:] *= 2.0
    wh_p[:, 6 * 128:] *= 2.0

    wx_aug = np.concatenate([kernel_p, bias_p[None]], 0).astype(BF16)  # [65,1024]
    # wh_sb[p, s, j] = wh_p[s*128+p, j]  (k-tile layout)
    wh_sb = np.ascontiguousarray(
        wh_p.reshape(2, 128, H4).transpose(1, 0, 2)
    ).astype(BF16)
    dw_sb = np.ascontiguousarray(
        np.asarray(dense_w, np.float32)[:, 0].reshape(2, 128).T
    ).astype(BF16)
    db = np.full((128, 1), float(np.asarray(dense_b).reshape(())), np.float32)

    # x cells: 0..15 real steps, 16..25 fake steps; transposed + ones row
    xcat = np.concatenate(
        [np.asarray(real_input, np.float32), np.asarray(fake_input, np.float32)],
        axis=1,
    )  # [B, 26, 64]
    xT = np.transpose(xcat, (1, 2, 0))  # [26, 64, B]
    xT = np.concatenate([xT, np.ones((NCELL, 1, B), np.float32)], axis=1)
    xT = xT.astype(BF16)  # [26, 65, B]

    in_maps = []
    for c in range(N_CORES):
        in_maps.append({
            "xT": np.ascontiguousarray(xT[:, :, c * BS:(c + 1) * BS]),
            "wx": wx_aug,
            "wh": wh_sb,
            "dw": dw_sb,
            "dbias": db,
        })
    return in_maps


_EXECS = {}


def _get_exec(loop_r=None):
    """Cached shard_map executable over the 8 cores (mirrors
    bass2jax.run_bass_via_pjrt but reusable across calls)."""
    if loop_r in _EXECS:
        return _EXECS[loop_r]

    import jax
    from jax.sharding import Mesh, PartitionSpec, NamedSharding
    from jax.experimental.shard_map import shard_map
    from concourse.bass2jax import (_bass_exec_p, install_neuronx_cc_hook,
                                    partition_id_tensor)

    install_neuronx_cc_hook()
    nc = _get_program(loop_r)

    partition_name = nc.partition_id_tensor.name if nc.partition_id_tensor else None
    in_names, out_names, out_avals, zero_outs = [], [], [], []
    for alloc in nc.m.functions[0].allocations:
        if not isinstance(alloc, mybir.MemoryLocationSet):
            continue
        name = alloc.memorylocations[0].name
        if alloc.kind == "ExternalInput":
            if name != partition_name:
                in_names.append(name)
        elif alloc.kind == "ExternalOutput":
            out_names.append(name)
            shape = tuple(alloc.tensor_shape)
            dtype = mybir.dt.np(alloc.dtype)
            out_avals.append(jax.core.ShapedArray(shape, dtype))
            zero_outs.append(np.zeros(shape, dtype))
    n_params = len(in_names)
    all_in_names = in_names + out_names
    if partition_name is not None:
        all_in_names = all_in_names + [partition_name]

    def _body(*args):
        operands = list(args)
        if partition_name is not None:
            operands.append(partition_id_tensor())
        outs = _bass_exec_p.bind(
            *operands,
            out_avals=tuple(out_avals),
            in_names=tuple(all_in_names),
            out_names=tuple(out_names),
            lowering_input_output_aliases=(),
            sim_require_finite=True,
            sim_require_nnan=True,
            nc=nc,
        )
        return tuple(outs)

    devices = jax.devices()[:N_CORES]
    mesh = Mesh(np.asarray(devices), ("core",))
    n_args = n_params + len(out_names)
    fn = jax.jit(
        shard_map(_body, mesh=mesh,
                  in_specs=(PartitionSpec("core"),) * n_args,
                  out_specs=(PartitionSpec("core"),) * len(out_names),
                  check_rep=False),
        keep_unused=True,
    )
    sharding = NamedSharding(mesh, PartitionSpec("core"))
    _EXECS[loop_r] = dict(fn=fn, in_names=in_names, out_names=out_names,
                          out_avals=out_avals, zero_outs=zero_outs,
                          sharding=sharding)
    return _EXECS[loop_r]


def _concat_args(ex, in_maps):
    args = [
        np.concatenate([np.asarray(m[name]) for m in in_maps], axis=0)
        for name in ex["in_names"]
    ]
    args += [
        np.zeros((N_CORES * z.shape[0], *z.shape[1:]), z.dtype)
        for z in ex["zero_outs"]
    ]
    return args


def _split_out(ex, out_arrs):
    stacked = np.asarray(out_arrs[0], np.float32).reshape(N_CORES, 2, PRED, BS)
    real = stacked[:, 0].transpose(0, 2, 1).reshape(B, PRED, 1)
    fake = stacked[:, 1].transpose(0, 2, 1).reshape(B, PRED, 1)
    return np.ascontiguousarray(real), np.ascontiguousarray(fake)


def run(inputs):
    """Run once; returns (real_pred, fake_pred)."""
    ex = _get_exec()
    in_maps = _prep_inputs(**inputs)
    out_arrs = ex["fn"](*_concat_args(ex, in_maps))
    return _split_out(ex, out_arrs)


def bench(inputs, iters=32):
    """Steady-state timing: device-resident args, async dispatch loop."""
    tn, _ = _bench_exec(None, inputs, iters)
    return tn, tn


def _bench_prep(loop_r, inputs):
    import jax

    ex = _get_exec(loop_r)
    in_maps = _prep_inputs(**inputs)
    args = [jax.device_put(a, ex["sharding"]) for a in _concat_args(ex, in_maps)]
    for a in args:
        a.block_until_ready()
    out = ex["fn"](*args)  # warmup / compile
    jax.block_until_ready(out)
    return ex, args


def bench_hw(inputs, r_hi=128, r_lo=8, samples=10):
    """Per-NEFF-iteration HW time via in-kernel For_i loop: INTERLEAVED
    (r_hi, r_lo) dispatch pairs -- the terminal's clock drifts on the
    scale of minutes, so block-mode (all r_hi then all r_lo) produces
    diffs off by +-30%; per-pair diffs taken seconds apart cancel the
    drift, and the median rejects stragglers. Alternating executables
    also forces a NEFF reload on every dispatch, which lands on both
    variants equally and cancels in the diff."""
    import jax
    import time

    def one(ex, args):
        t0 = time.perf_counter()
        out = ex["fn"](*args)
        jax.block_until_ready(out)
        return time.perf_counter() - t0

    ex_hi, args_hi = _bench_prep(r_hi, inputs)
    ex_lo, args_lo = _bench_prep(r_lo, inputs)
    one(ex_hi, args_hi)
    one(ex_lo, args_lo)  # absorb first NEFF switches
    diffs = []
    for _ in range(samples):
        t_hi = one(ex_hi, args_hi)
        t_lo = one(ex_lo, args_lo)
        diffs.append((t_hi - t_lo) / (r_hi - r_lo))
    diffs.sort()
    return diffs[len(diffs) // 2], diffs[0], diffs[-1]


def kernel(real_input, fake_input, kernel, recurrent_kernel, bias, dense_w,
           dense_b):
    return run(dict(
        real_input=real_input, fake_input=fake_input, kernel=kernel,
        recurrent_kernel=recurrent_kernel, bias=bias, dense_w=dense_w,
        dense_b=dense_b,
    ))


# revision 51
# speedup vs baseline: 1.0744x; 1.0744x over previous
"""Trainium2 Bass kernel for the LSTM GAN-discriminator problem.

Math (reference): two 16-step LSTM passes over [B=4096, T=16, F=64] sharing the
first PREV=6 steps (fake sequence = real[:, :6] ++ fake_input), then a dense+
sigmoid head on hidden states of steps 6..15 of each pass.

Strategy:
  - Data parallel: batch 4096 -> 8 cores x 512 rows; weights replicated.
  - Shared prefix: 6 cells at N=512, then the two branches run as separate
    interleaved chains (10 real + 10 fake cells), 26 cell evaluations total.
  - Transposed layout: features on partitions, batch on the free dim. The
    4H=1024 gate columns live as 8 "banks" of 128 partitions; hidden state
    h lives as [128, 2, N] (2 k-tiles of 128 features), so the recurrent
    matmul contracts K=128 per k-tile with NO per-step transpose.
  - Bias is folded into the x-projection via an augmented ones-row (K=65).
  - All matmul operands bf16 (PSUM accumulates fp32); gates/c/h bf16 for
    the 2x DVE tensor_tensor mode. (fp8 DoubleRow for the recurrent matmul
    was tried twice: the non-FWL 256-column LDWEIGHTS erases the matmul
    saving and the 1x-rate fp8-out DVE h-op lengthens the recurrence
    chain -- HW-slower both times. bf16 kept.)
  - ACT is the co-bottleneck with PE (~5.1us/cell each), so ACT work is
    minimized structurally:
      * g-gate tanh becomes sigmoid: the g columns of wx/wh/bias are
        pre-doubled on the host (tanh(z) = 2*sigmoid(2z) - 1), so PSUM
        splits as [i,i,f,f] / [o,o,g,g] (4+4 banks, bufs=1) and ONE
        2048-col sigmoid drains each tile -- 2 gate ACT ops per cell
        instead of 3, and a tile's drain latency stays under the PE
        period (a 6+2 split with a 3072-col sigmoid stalls the next
        cell's matmuls on the drain).
      * The c-update absorbs the g rescale off the serial chain:
        c = f*c_prev + i*(2*sg-1) is computed as (f*c_prev - i) + (2i)*sg,
        so only two DVE ops sit between sigmoid([o,g]) and tanh(c).
      * Dense head: steps packed 4-at-a-time (4+4+2) into a 4-bank PSUM
        tile -- M=1 matmuls col-packed via tile_position into partitions
        {0 (real), 32 (fake)}, steps on free quarters -- so THREE
        sigmoids total drain the whole head. (Do not use col-groups
        (0,64)/(0,96): quadrant-3 tile_position wedged the device.)
  - Cell emission is phase-split: phase 1 (matmuls + 2 sigmoids), phase 2
    (DVE combine + tanh(c) + h) is emitted one cell-chunk LATE so tanh(c),
    which waits on the DVE chain, cannot head-of-line-block the next
    cell's already-ready sigmoids in the strict-FIFO ACT queue.
  - A PE warm-up burst (8 matmuls on a zeroed scratch tile) runs during
    the initial weight DMAs so the PE_HAM clock gate reaches 8/8 (2.4 GHz)
    ~10us into the kernel instead of ~23us.
  - Prefix cells run as 2 batch chunks of 256 (pipeline depth for the
    serial chain); branch cells run one chunk of 512, with the two
    branches alternating cells to keep PE/ACT/DVE overlapped.
  - DO NOT reorder a cell's matmuls x-passes-first across banks: the
    interleaved PSUM accumulation groups (start on all 8 banks before any
    stop) produced a 20x accuracy regression on HW. Keep the per-bank
    x, h0, h1 triplets.
  - Measured end-to-end rel err ~4.1e-4 (tolerance 2e-2).
"""

import sys

if "/opt/trn_rl_repo" not in sys.path:
    sys.path.insert(0, "/opt/trn_rl_repo")

import numpy as np
import ml_dtypes

import concourse.mybir as mybir
import concourse.tile as tile
from concourse import bacc

BF16 = ml_dtypes.bfloat16

PREV, PRED, FEAT, HID = 6, 10, 64, 256
B = 4096
N_CORES = 8
BS = B // N_CORES          # 512 rows per core
CH = 256                   # chunk of the per-core batch
NCH = BS // CH             # 2 chunks
NCELL = PREV + 2 * PRED    # 26 cell evaluations per core
H4 = 4 * HID               # 1024

# gate bank order [i_s0, i_s1, f_s0, f_s1, o_s0, o_s1, g_s0, g_s1]
# (original z column order is i, f, g, o)
_GATE_BASE = [0, 0, 256, 256, 768, 768, 512, 512]
PERM = np.concatenate(
    [np.arange(_GATE_BASE[m] + 128 * (m % 2), _GATE_BASE[m] + 128 * (m % 2) + 128)
     for m in range(8)]
)

# canonical cell ids: 0..5 prefix, 6..15 real steps 6..15, 16..25 fake steps 6..15
# processing order interleaves the two independent branches; the fake cell
# goes first in each pair so f6 can read c5 out of c_real before r6
# overwrites it (no c copy needed at the branch point)
CELL_ORDER = list(range(PREV)) + [
    c for t in range(PRED) for c in (PREV + PRED + t, PREV + t)
]


def _h_src(hid_):
    """canonical id of the cell whose h feeds this cell (None for cell 0)."""
    if hid_ == 0:
        return None
    if hid_ == PREV + PRED:  # first fake cell branches off the prefix
        return PREV - 1
    return hid_ - 1


def _build_program(loop_r=None):
    f32 = mybir.dt.float32
    bf16 = mybir.dt.bfloat16
    AF = mybir.ActivationFunctionType
    OP = mybir.AluOpType

    nc = bacc.Bacc("TRN2", target_bir_lowering=False, debug=False,
                   num_devices=N_CORES)

    xT = nc.dram_tensor("xT", [NCELL, FEAT + 1, BS], bf16, kind="ExternalInput").ap()
    wx = nc.dram_tensor("wx", [FEAT + 1, H4], bf16, kind="ExternalInput").ap()
    wh = nc.dram_tensor("wh", [128, 2, H4], bf16, kind="ExternalInput").ap()
    dw = nc.dram_tensor("dw", [128, 2], bf16, kind="ExternalInput").ap()
    dbias = nc.dram_tensor("dbias", [128, 1], mybir.dt.float32,
                           kind="ExternalInput").ap()
    outT = nc.dram_tensor("outT", [2, PRED, BS], f32, kind="ExternalOutput").ap()

    with tile.TileContext(nc) as tc:
        with (
            tc.tile_pool(name="const", bufs=1) as const,
            tc.tile_pool(name="xpool", bufs=4) as xpool,
            tc.tile_pool(name="zpool", bufs=2, space="PSUM") as zpool,
            tc.tile_pool(name="gpool", bufs=3) as gpool,
            tc.tile_pool(name="tpool", bufs=3) as tpool,
        ):
            wx_t = const.tile([FEAT + 1, H4], bf16, tag="wx")
            wh_t = const.tile([128, 2, H4], bf16, tag="wh")
            dw_t = const.tile([128, 2], bf16, tag="dw")
            db_t = const.tile([128, 1], f32, tag="db")
            c_real = const.tile([128, H4], bf16, tag="c_real")
            c_fake = const.tile([128, H4], bf16, tag="c_fake")
            dsig = const.tile([33, PRED * BS], f32, tag="dsig")
            h_tiles = [const.tile([128, 2, BS], bf16, tag=f"h{i}", name=f"h{i}")
                       for i in range(NCELL)]

            # (A PE warm-up matmul burst was tried here to flip the PE_HAM
            # clock gate early; traces showed the flip timestamp unchanged
            # (~22us) -- the prefix phase is latency-bound and stays at
            # K=4/8 regardless -- so it was only a ~3.5us startup delay.
            # Also A/B-tested and rejected: an 8-bank manually-addressed
            # PSUM region with prefix chunk ping-pong (neutral: the prefix
            # PE stalls it removes are symptoms -- ACT/chain is critical
            # there), and merging the prefix chunk's za|zb into one
            # 2048-col sigmoid (+7us: the prefix is recurrence-latency-
            # bound and the merge delays i/f gate availability ~1.5us).

            # dummy activation: forces the sigmoid/tanh ACT table load to
            # happen during the weight DMAs instead of on the critical path
            warm = tpool.tile([128, 1], f32, tag="warm", name="warm")
            nc.scalar.activation(warm[:, :], db_t[:, :], AF.Sigmoid)
            nc.scalar.activation(warm[:, :], db_t[:, :], AF.Tanh)

            nc.sync.dma_start(wx_t[:, :], wx)
            nc.sync.dma_start(wh_t[:, :, :], wh)
            nc.sync.dma_start(dw_t[:, :], dw)
            nc.sync.dma_start(db_t[:, :], dbias)

            def _nch(hid_):
                return NCH if hid_ < PREV else 1

            gstate = {}

            def emit_p1(hid_, ch, x_t):
                """Phase 1 of one cell-chunk: gate matmuls + the two
                sigmoids. Prefix cells run as 2 batch chunks of 256
                (pipeline depth for the serial chain); branch cells run one
                chunk of 512 (alternating real/fake cells give the pipeline
                parallelism instead). PSUM is split [i,f] / [o,g] (4+4
                banks at N=512, bufs=1 each) so the next cell's matmuls can
                reuse a tile as soon as its sigmoid drains it."""
                h_prev = None if _h_src(hid_) is None else h_tiles[_h_src(hid_)]
                cw = BS // _nch(hid_)

                za = zpool.tile([128, 4 * cw], f32, tag="ta", name="za", bufs=1)
                zb = zpool.tile([128, 4 * cw], f32, tag="tb", name="zb", bufs=1)
                banks = [(za, m, m) for m in range(4)] + \
                        [(zb, m, m + 4) for m in range(4)]
                for ztile, k, m in banks:
                    zs = ztile[:, k * cw:(k + 1) * cw]
                    nc.tensor.matmul(
                        zs,
                        wx_t[:, m * 128:(m + 1) * 128],
                        x_t[:, ch * cw:(ch + 1) * cw],
                        start=True, stop=(h_prev is None),
                    )
                    if h_prev is not None:
                        for s in range(2):
                            nc.tensor.matmul(
                                zs,
                                wh_t[:, s, m * 128:(m + 1) * 128],
                                h_prev[:, s, ch * cw:(ch + 1) * cw],
                                start=False, stop=(s == 1),
                            )
                # --- ACT: gates (bank order [i i f f] / [o o g g]). The g
                # columns of wx/wh/bias are pre-doubled on the host so
                # tanh(zg) = 2*sigmoid(2*zg) - 1: ONE sigmoid covers
                # [o o g g]; a cheap DVE fixup rescales g. ---
                ga = gpool.tile([128, 4 * cw], bf16, tag="ga", name="ga")
                gb = gpool.tile([128, 4 * cw], bf16, tag="gb", name="gb")
                # ONE sigmoid per PSUM tile. (Splitting sigmoid(za) into
                # two 1024-col halves to free the za banks earlier was
                # A/B-tested: +15us WORSE -- the second op's fixed ~350-cyc
                # overhead sits on the drain path, so the tile frees later.)
                nc.scalar.activation(ga[:, :], za[:, :], AF.Sigmoid)
                nc.scalar.activation(gb[:, :], zb[:, :], AF.Sigmoid)
                gstate[(hid_, ch)] = (ga, gb)

            def emit_p2(hid_, ch):
                """Phase 2 of one cell-chunk: DVE gate combine + tanh(c) +
                h. Emitted one cell-chunk BEHIND phase 1 so this chunk's
                tanh(c) (which waits on the DVE chain) enqueues on the ACT
                FIFO after the next chunk's sigmoids -- the strict-FIFO ACT
                queue would otherwise head-of-line block on it."""
                ga, gb = gstate.pop((hid_, ch))
                h_prev = None if _h_src(hid_) is None else h_tiles[_h_src(hid_)]
                c_in = c_out = c_real if hid_ < PREV + PRED else c_fake
                if hid_ == PREV + PRED:
                    c_in = c_real  # branch point: fake chain starts from c5
                nch = _nch(hid_)
                cw = BS // nch

                def v3(t, lo):
                    return t[:, lo:lo + 2 * cw].rearrange(
                        "p (s n) -> p s n", s=2, n=cw)

                i3, f3 = v3(ga, 0), v3(ga, 2 * cw)
                o3, sg3 = v3(gb, 0), v3(gb, 2 * cw)

                def cvw(t):
                    return t[:, :].rearrange("p (s c n) -> p s c n",
                                             s=2, c=nch, n=cw)[:, :, ch]

                cvi, cvo = cvw(c_in), cvw(c_out)
                # c = f*c_prev + i*(2*sg - 1) reassociated as
                #     (f*c_prev - i) + (2i)*sg
                # so everything except the last two DVE ops depends only on
                # the EARLIER sigmoid (za): the serial chain after sigmoid(zb)
                # stays two ops long, same as with a native tanh g-gate.
                i2 = tpool.tile([128, 2 * cw], bf16, tag="g2", name="i2")
                vv = tpool.tile([128, 2 * cw], bf16, tag="vv", name="vv")
                nc.vector.tensor_scalar_mul(v3(i2, 0), i3, 2.0)
                if h_prev is None:
                    nc.vector.tensor_tensor(v3(vv, 0), v3(i2, 0), sg3, OP.mult)
                    nc.vector.tensor_tensor(cvo, v3(vv, 0), i3, OP.subtract)
                else:
                    fc = tpool.tile([128, 2 * cw], bf16, tag="fc", name="fc")
                    ig = tpool.tile([128, 2 * cw], bf16, tag="ig", name="ig")
                    nc.vector.tensor_tensor(v3(fc, 0), f3, cvi, OP.mult)
                    nc.vector.tensor_tensor(v3(ig, 0), v3(fc, 0), i3, OP.subtract)
                    nc.vector.tensor_tensor(v3(vv, 0), v3(i2, 0), sg3, OP.mult)
                    nc.vector.tensor_tensor(cvo, v3(ig, 0), v3(vv, 0), OP.add)
                tcn = tpool.tile([128, 2 * cw], bf16, tag="tc", name="tc")
                nc.scalar.activation(v3(tcn, 0), cvo, AF.Tanh)
                hv = h_tiles[hid_][:, :, ch * cw:(ch + 1) * cw]
                nc.vector.tensor_tensor(hv, o3, v3(tcn, 0), OP.mult)

            def emit_dense(q_):
                """pred[:, 4q:4q+4] for both branches in ONE 4-bank PSUM
                tile: partitions {0 real, 32 fake} via tile_position, the 4
                steps of the quad on free quarters, so a single
                sigmoid(+bias) drains it. 3 emissions total (4+4+2 steps).
                (2-step pairs were A/B-tested: consistently ~4us slower
                single-shot -- the extra sigmoids and extra PSUM-chain
                disruptions outweigh the shorter matmul bursts.)"""
                nstep = min(4, PRED - 4 * q_)
                dp = zpool.tile([128, nstep * BS], f32, tag="ta", name="dp",
                                bufs=1)
                for dt_ in range(nstep):
                    for br, j0 in ((0, 0), (1, 32)):
                        cell = (PREV if br == 0 else PREV + PRED) + 4 * q_ + dt_
                        for s in range(2):
                            nc.tensor.matmul(
                                dp[j0:j0 + 1, dt_ * BS:(dt_ + 1) * BS],
                                dw_t[:, s:s + 1],
                                h_tiles[cell][:, s, :],
                                start=(s == 0), stop=(s == 1),
                                tile_position=(0, j0),
                            )
                nc.scalar.activation(
                    dsig[:, 4 * q_ * BS:(4 * q_ + nstep) * BS],
                    dp[0:33, 0:nstep * BS],
                    AF.Sigmoid, bias=db_t[0:33, 0:1])

            def emit_body():
              units = []
              for hid_ in CELL_ORDER:
                  for ch in range(_nch(hid_)):
                      units.append((hid_, ch))

              x_tiles = {}
              pending = None
              for hid_, ch in units:
                if ch == 0:
                    x_t = xpool.tile([FEAT + 1, BS], bf16, tag="x", name="x")
                    nc.sync.dma_start(x_t[:, :], xT[hid_])
                    x_tiles[hid_] = x_t

                if hid_ == PREV + PRED and pending is not None:
                    # branch point: the first fake cell reads BOTH chunks of
                    # h5, so the delayed phase-2 of (5, 1) must land first
                    emit_p2(*pending)
                    pending = None

                emit_p1(hid_, ch, x_tiles[hid_])
                if pending is not None:
                    emit_p2(*pending)
                pending = (hid_, ch)

                t_r = hid_ - PREV  # real cell completes step t_r
                if PREV <= hid_ < PREV + PRED and t_r in (5, 9):
                    # dense quad (4q..4q+3), emitted behind the scan
                    emit_dense((t_r - 5) // 4)

              emit_p2(*pending)
              emit_dense(2)

              nc.sync.dma_start(outT[0], dsig[0:1, :])
              nc.sync.dma_start(outT[1], dsig[32:33, :])

            if loop_r is None:
                emit_body()
            else:
                with tc.For_i(0, loop_r, 1,
                              hint_engines=(mybir.EngineType.PE,)):
                    emit_body()

    nc.compile()
    return nc


_PROGRAMS = {}


def _get_program(loop_r=None):
    if loop_r not in _PROGRAMS:
        _PROGRAMS[loop_r] = _build_program(loop_r)
    return _PROGRAMS[loop_r]


def _prep_inputs(real_input, fake_input, kernel, recurrent_kernel, bias, dense_w,
                 dense_b):
    kernel_p = np.asarray(kernel, np.float32)[:, PERM]
    bias_p = np.asarray(bias, np.float32)[PERM]
    wh_p = np.asarray(recurrent_kernel, np.float32)[:, PERM]
    # double the g-gate columns (permuted banks 6,7): tanh(z) = 2*sig(2z)-1
    kernel_p[:, 6 * 128:] *= 2.0
    bias_p[6 * 128:] *= 2.0
    wh_p[:, 6 * 128:] *= 2.0

    wx_aug = np.concatenate([kernel_p, bias_p[None]], 0).astype(BF16)  # [65,1024]
    # wh_sb[p, s, j] = wh_p[s*128+p, j]  (k-tile layout)
    wh_sb = np.ascontiguousarray(
        wh_p.reshape(2, 128, H4).transpose(1, 0, 2)
    ).astype(BF16)
    dw_sb = np.ascontiguousarray(
        np.asarray(dense_w, np.float32)[:, 0].reshape(2, 128).T
    ).astype(BF16)
    db = np.full((128, 1), float(np.asarray(dense_b).reshape(())), np.float32)

    # x cells: 0..15 real steps, 16..25 fake steps; transposed + ones row
    xcat = np.concatenate(
        [np.asarray(real_input, np.float32), np.asarray(fake_input, np.float32)],
        axis=1,
    )  # [B, 26, 64]
    xT = np.transpose(xcat, (1, 2, 0))  # [26, 64, B]
    xT = np.concatenate([xT, np.ones((NCELL, 1, B), np.float32)], axis=1)
    xT = xT.astype(BF16)  # [26, 65, B]

    in_maps = []
    for c in range(N_CORES):
        in_maps.append({
            "xT": np.ascontiguousarray(xT[:, :, c * BS:(c + 1) * BS]),
            "wx": wx_aug,
            "wh": wh_sb,
            "dw": dw_sb,
            "dbias": db,
        })
    return in_maps


_EXECS = {}


def _get_exec(loop_r=None):
    """Cached shard_map executable over the 8 cores (mirrors
    bass2jax.run_bass_via_pjrt but reusable across calls)."""
    if loop_r in _EXECS:
        return _EXECS[loop_r]

    import jax
    from jax.sharding import Mesh, PartitionSpec, NamedSharding
    from jax.experimental.shard_map import shard_map
    from concourse.bass2jax import (_bass_exec_p, install_neuronx_cc_hook,
                                    partition_id_tensor)

    install_neuronx_cc_hook()
    nc = _get_program(loop_r)

    partition_name = nc.partition_id_tensor.name if nc.partition_id_tensor else None
    in_names, out_names, out_avals, zero_outs = [], [], [], []
    for alloc in nc.m.functions[0].allocations:
        if not isinstance(alloc, mybir.MemoryLocationSet):
            continue
        name = alloc.memorylocations[0].name
        if alloc.kind == "ExternalInput":
            if name != partition_name:
                in_names.append(name)
        elif alloc.kind == "ExternalOutput":
            out_names.append(name)
            shape = tuple(alloc.tensor_shape)
            dtype = mybir.dt.np(alloc.dtype)
            out_avals.append(jax.core.ShapedArray(shape, dtype))
            zero_outs.append(np.zeros(shape, dtype))
    n_params = len(in_names)
    all_in_names = in_names + out_names
    if partition_name is not None:
        all_in_names = all_in_names + [partition_name]

    def _body(*args):
        operands = list(args)
        if partition_name is not None:
            operands.append(partition_id_tensor())
        outs = _bass_exec_p.bind(
            *operands,
            out_avals=tuple(out_avals),
            in_names=tuple(all_in_names),
            out_names=tuple(out_names),
            lowering_input_output_aliases=(),
            sim_require_finite=True,
            sim_require_nnan=True,
            nc=nc,
        )
        return tuple(outs)

    devices = jax.devices()[:N_CORES]
    mesh = Mesh(np.asarray(devices), ("core",))
    n_args = n_params + len(out_names)
    fn = jax.jit(
        shard_map(_body, mesh=mesh,
                  in_specs=(PartitionSpec("core"),) * n_args,
                  out_specs=(PartitionSpec("core"),) * len(out_names),
                  check_rep=False),
        keep_unused=True,
    )
    sharding = NamedSharding(mesh, PartitionSpec("core"))
    _EXECS[loop_r] = dict(fn=fn, in_names=in_names, out_names=out_names,
                          out_avals=out_avals, zero_outs=zero_outs,
                          sharding=sharding)
    return _EXECS[loop_r]


def _concat_args(ex, in_maps):
    args = [
        np.concatenate([np.asarray(m[name]) for m in in_maps], axis=0)
        for name in ex["in_names"]
    ]
    args += [
        np.zeros((N_CORES * z.shape[0], *z.shape[1:]), z.dtype)
        for z in ex["zero_outs"]
    ]
    return args


def _split_out(ex, out_arrs):
    stacked = np.asarray(out_arrs[0], np.float32).reshape(N_CORES, 2, PRED, BS)
    real = stacked[:, 0].transpose(0, 2, 1).reshape(B, PRED, 1)
    fake = stacked[:, 1].transpose(0, 2, 1).reshape(B, PRED, 1)
    return np.ascontiguousarray(real), np.ascontiguousarray(fake)


def run(inputs):
    """Run once; returns (real_pred, fake_pred)."""
    ex = _get_exec()
    in_maps = _prep_inputs(**inputs)
    out_arrs = ex["fn"](*_concat_args(ex, in_maps))
    return _split_out(ex, out_arrs)


def bench(inputs, iters=32):
    """Steady-state timing: device-resident args, async dispatch loop."""
    tn, _ = _bench_exec(None, inputs, iters)
    return tn, tn


def _bench_prep(loop_r, inputs):
    import jax

    ex = _get_exec(loop_r)
    in_maps = _prep_inputs(**inputs)
    args = [jax.device_put(a, ex["sharding"]) for a in _concat_args(ex, in_maps)]
    for a in args:
        a.block_until_ready()
    out = ex["fn"](*args)  # warmup / compile
    jax.block_until_ready(out)
    return ex, args


def bench_hw(inputs, r_hi=128, r_lo=8, samples=10):
    """Per-NEFF-iteration HW time via in-kernel For_i loop: INTERLEAVED
    (r_hi, r_lo) dispatch pairs -- the terminal's clock drifts on the
    scale of minutes, so block-mode (all r_hi then all r_lo) produces
    diffs off by +-30%; per-pair diffs taken seconds apart cancel the
    drift, and the median rejects stragglers. Alternating executables
    also forces a NEFF reload on every dispatch, which lands on both
    variants equally and cancels in the diff."""
    import jax
    import time

    def one(ex, args):
        t0 = time.perf_counter()
        out = ex["fn"](*args)
        jax.block_until_ready(out)
        return time.perf_counter() - t0

    ex_hi, args_hi = _bench_prep(r_hi, inputs)
    ex_lo, args_lo = _bench_prep(r_lo, inputs)
    one(ex_hi, args_hi)
    one(ex_lo, args_lo)  # absorb first NEFF switches
    diffs = []
    for _ in range(samples):
        t_hi = one(ex_hi, args_hi)
        t_lo = one(ex_lo, args_lo)
        diffs.append((t_hi - t_lo) / (r_hi - r_lo))
    diffs.sort()
    return diffs[len(diffs) // 2], diffs[0], diffs[-1]


def kernel(real_input, fake_input, kernel, recurrent_kernel, bias, dense_w,
           dense_b):
    return run(dict(
        real_input=real_input, fake_input=fake_input, kernel=kernel,
        recurrent_kernel=recurrent_kernel, bias=bias, dense_w=dense_w,
        dense_b=dense_b,
    ))


# revision 52
# speedup vs baseline: 1.2486x; 1.1621x over previous
"""Trainium2 Bass kernel for the LSTM GAN-discriminator problem.

Math (reference): two 16-step LSTM passes over [B=4096, T=16, F=64] sharing the
first PREV=6 steps (fake sequence = real[:, :6] ++ fake_input), then a dense+
sigmoid head on hidden states of steps 6..15 of each pass.

Strategy:
  - Data parallel: batch 4096 -> 8 cores x 512 rows; weights replicated.
  - Shared prefix: 6 cells at N=512, then the two branches run as separate
    interleaved chains (10 real + 10 fake cells), 26 cell evaluations total.
  - Transposed layout: features on partitions, batch on the free dim. The
    4H=1024 gate columns live as 8 "banks" of 128 partitions; hidden state
    h lives as [128, 2, N] (2 k-tiles of 128 features), so the recurrent
    matmul contracts K=128 per k-tile with NO per-step transpose.
  - Bias is folded into the x-projection via an augmented ones-row (K=65).
  - All matmul operands bf16 (PSUM accumulates fp32); gates/c/h bf16 for
    the 2x DVE tensor_tensor mode. (fp8 DoubleRow for the recurrent matmul
    was tried twice: the non-FWL 256-column LDWEIGHTS erases the matmul
    saving and the 1x-rate fp8-out DVE h-op lengthens the recurrence
    chain -- HW-slower both times. bf16 kept.)
  - ACT is the co-bottleneck with PE (~5.1us/cell each), so ACT work is
    minimized structurally:
      * g-gate tanh becomes sigmoid: the g columns of wx/wh/bias are
        pre-doubled on the host (tanh(z) = 2*sigmoid(2z) - 1), so PSUM
        splits as [i,i,f,f] / [o,o,g,g] (4+4 banks, bufs=1) and ONE
        2048-col sigmoid drains each tile -- 2 gate ACT ops per cell
        instead of 3, and a tile's drain latency stays under the PE
        period (a 6+2 split with a 3072-col sigmoid stalls the next
        cell's matmuls on the drain).
      * The c-update absorbs the g rescale off the serial chain:
        c = f*c_prev + i*(2*sg-1) is computed as (f*c_prev - i) + (2i)*sg,
        so only two DVE ops sit between sigmoid([o,g]) and tanh(c).
      * Dense head: steps packed 4-at-a-time (4+4+2) into a 4-bank PSUM
        tile -- M=1 matmuls col-packed via tile_position into partitions
        {0 (real), 32 (fake)}, steps on free quarters -- so THREE
        sigmoids total drain the whole head. (Do not use col-groups
        (0,64)/(0,96): quadrant-3 tile_position wedged the device.)
  - Cell emission is phase-split: phase 1 (matmuls + 2 sigmoids), phase 2
    (DVE combine + tanh(c) + h) is emitted one cell-chunk LATE so tanh(c),
    which waits on the DVE chain, cannot head-of-line-block the next
    cell's already-ready sigmoids in the strict-FIFO ACT queue.
  - A PE warm-up burst (8 matmuls on a zeroed scratch tile) runs during
    the initial weight DMAs so the PE_HAM clock gate reaches 8/8 (2.4 GHz)
    ~10us into the kernel instead of ~23us.
  - Prefix cells run as 2 batch chunks of 256 (pipeline depth for the
    serial chain); branch cells run one chunk of 512, with the two
    branches alternating cells to keep PE/ACT/DVE overlapped.
  - DO NOT reorder a cell's matmuls x-passes-first across banks: the
    interleaved PSUM accumulation groups (start on all 8 banks before any
    stop) produced a 20x accuracy regression on HW. Keep the per-bank
    x, h0, h1 triplets.
  - Measured end-to-end rel err ~4.1e-4 (tolerance 2e-2).
"""

import sys

if "/opt/trn_rl_repo" not in sys.path:
    sys.path.insert(0, "/opt/trn_rl_repo")

import numpy as np
import ml_dtypes

import concourse.mybir as mybir
import concourse.tile as tile
from concourse import bacc

BF16 = ml_dtypes.bfloat16

PREV, PRED, FEAT, HID = 6, 10, 64, 256
B = 4096
N_CORES = 8
BS = B // N_CORES          # 512 rows per core
CH = 256                   # chunk of the per-core batch
NCH = BS // CH             # 2 chunks
NCELL = PREV + 2 * PRED    # 26 cell evaluations per core
H4 = 4 * HID               # 1024

# gate bank order [i_s0, i_s1, f_s0, f_s1, o_s0, o_s1, g_s0, g_s1]
# (original z column order is i, f, g, o)
_GATE_BASE = [0, 0, 256, 256, 768, 768, 512, 512]
PERM = np.concatenate(
    [np.arange(_GATE_BASE[m] + 128 * (m % 2), _GATE_BASE[m] + 128 * (m % 2) + 128)
     for m in range(8)]
)

# canonical cell ids: 0..5 prefix, 6..15 real steps 6..15, 16..25 fake steps 6..15
# processing order interleaves the two independent branches; the fake cell
# goes first in each pair so f6 can read c5 out of c_real before r6
# overwrites it (no c copy needed at the branch point)
CELL_ORDER = list(range(PREV)) + [
    c for t in range(PRED) for c in (PREV + PRED + t, PREV + t)
]


def _h_src(hid_):
    """canonical id of the cell whose h feeds this cell (None for cell 0)."""
    if hid_ == 0:
        return None
    if hid_ == PREV + PRED:  # first fake cell branches off the prefix
        return PREV - 1
    return hid_ - 1


def _build_program(loop_r=None):
    f32 = mybir.dt.float32
    bf16 = mybir.dt.bfloat16
    AF = mybir.ActivationFunctionType
    OP = mybir.AluOpType

    nc = bacc.Bacc("TRN2", target_bir_lowering=False, debug=False,
                   num_devices=N_CORES)

    xT = nc.dram_tensor("xT", [NCELL, FEAT + 1, BS], bf16, kind="ExternalInput").ap()
    wx = nc.dram_tensor("wx", [FEAT + 1, H4], bf16, kind="ExternalInput").ap()
    wh = nc.dram_tensor("wh", [128, 2, H4], bf16, kind="ExternalInput").ap()
    dw = nc.dram_tensor("dw", [128, 2], bf16, kind="ExternalInput").ap()
    dbias = nc.dram_tensor("dbias", [128, 1], mybir.dt.float32,
                           kind="ExternalInput").ap()
    outT = nc.dram_tensor("outT", [2, PRED, BS], f32, kind="ExternalOutput").ap()

    with tile.TileContext(nc) as tc:
        with (
            tc.tile_pool(name="const", bufs=1) as const,
            tc.tile_pool(name="xpool", bufs=4) as xpool,
            tc.tile_pool(name="zpool", bufs=2, space="PSUM") as zpool,
            tc.tile_pool(name="gpool", bufs=3) as gpool,
            tc.tile_pool(name="tpool", bufs=3) as tpool,
        ):
            wx_t = const.tile([FEAT + 1, H4], bf16, tag="wx")
            wh_t = const.tile([128, 2, H4], bf16, tag="wh")
            dw_t = const.tile([128, 2], bf16, tag="dw")
            db_t = const.tile([128, 1], f32, tag="db")
            c_real = const.tile([128, H4], bf16, tag="c_real")
            c_fake = const.tile([128, H4], bf16, tag="c_fake")
            dsig = const.tile([33, PRED * BS], f32, tag="dsig")
            h_tiles = [const.tile([128, 2, BS], bf16, tag=f"h{i}", name=f"h{i}")
                       for i in range(NCELL)]

            # (A PE warm-up matmul burst was tried here to flip the PE_HAM
            # clock gate early; traces showed the flip timestamp unchanged
            # (~22us) -- the prefix phase is latency-bound and stays at
            # K=4/8 regardless -- so it was only a ~3.5us startup delay.
            # Also A/B-tested and rejected: an 8-bank manually-addressed
            # PSUM region with prefix chunk ping-pong (neutral: the prefix
            # PE stalls it removes are symptoms -- ACT/chain is critical
            # there), and merging the prefix chunk's za|zb into one
            # 2048-col sigmoid (+7us: the prefix is recurrence-latency-
            # bound and the merge delays i/f gate availability ~1.5us).

            # dummy activation: forces the sigmoid/tanh ACT table load to
            # happen during the weight DMAs instead of on the critical path
            warm = tpool.tile([128, 1], f32, tag="warm", name="warm")
            nc.scalar.activation(warm[:, :], db_t[:, :], AF.Sigmoid)
            nc.scalar.activation(warm[:, :], db_t[:, :], AF.Tanh)

            # weight DMAs sliced by gate-bank columns: cell 0's bank-m
            # matmul then waits only on its own slice (range-granular
            # deps), and the slices spread across idle DMA queues instead
            # of serializing on one -- the whole-tensor DMAs put the first
            # sigmoid ~8us into the kernel
            for j in range(4):
                nc.sync.dma_start(wx_t[:, j * 256:(j + 1) * 256],
                                  wx[:, j * 256:(j + 1) * 256])
            for s in range(2):
                for j in range(2):
                    nc.sync.dma_start(wh_t[:, s, j * 512:(j + 1) * 512],
                                      wh[:, s, j * 512:(j + 1) * 512])
            nc.sync.dma_start(dw_t[:, :], dw)
            nc.sync.dma_start(db_t[:, :], dbias)

            def _nch(hid_):
                return NCH if hid_ < PREV else 1

            gstate = {}

            def emit_p1(hid_, ch, x_t):
                """Phase 1 of one cell-chunk: gate matmuls + the two
                sigmoids. Prefix cells run as 2 batch chunks of 256
                (pipeline depth for the serial chain); branch cells run one
                chunk of 512 (alternating real/fake cells give the pipeline
                parallelism instead). PSUM is split [i,f] / [o,g] (4+4
                banks at N=512, bufs=1 each) so the next cell's matmuls can
                reuse a tile as soon as its sigmoid drains it."""
                h_prev = None if _h_src(hid_) is None else h_tiles[_h_src(hid_)]
                cw = BS // _nch(hid_)

                za = zpool.tile([128, 4 * cw], f32, tag="ta", name="za", bufs=1)
                zb = zpool.tile([128, 4 * cw], f32, tag="tb", name="zb", bufs=1)
                banks = [(za, m, m) for m in range(4)] + \
                        [(zb, m, m + 4) for m in range(4)]
                for ztile, k, m in banks:
                    zs = ztile[:, k * cw:(k + 1) * cw]
                    nc.tensor.matmul(
                        zs,
                        wx_t[:, m * 128:(m + 1) * 128],
                        x_t[:, ch * cw:(ch + 1) * cw],
                        start=True, stop=(h_prev is None),
                    )
                    if h_prev is not None:
                        for s in range(2):
                            nc.tensor.matmul(
                                zs,
                                wh_t[:, s, m * 128:(m + 1) * 128],
                                h_prev[:, s, ch * cw:(ch + 1) * cw],
                                start=False, stop=(s == 1),
                            )
                # --- ACT: gates (bank order [i i f f] / [o o g g]). The g
                # columns of wx/wh/bias are pre-doubled on the host so
                # tanh(zg) = 2*sigmoid(2*zg) - 1: ONE sigmoid covers
                # [o o g g]; a cheap DVE fixup rescales g. ---
                ga = gpool.tile([128, 4 * cw], bf16, tag="ga", name="ga")
                gb = gpool.tile([128, 4 * cw], bf16, tag="gb", name="gb")
                # ONE sigmoid per PSUM tile. (Splitting sigmoid(za) into
                # two 1024-col halves to free the za banks earlier was
                # A/B-tested: +15us WORSE -- the second op's fixed ~350-cyc
                # overhead sits on the drain path, so the tile frees later.)
                nc.scalar.activation(ga[:, :], za[:, :], AF.Sigmoid)
                nc.scalar.activation(gb[:, :], zb[:, :], AF.Sigmoid)
                gstate[(hid_, ch)] = (ga, gb)

            def emit_p2(hid_, ch):
                """Phase 2 of one cell-chunk: DVE gate combine + tanh(c) +
                h. Emitted one cell-chunk BEHIND phase 1 so this chunk's
                tanh(c) (which waits on the DVE chain) enqueues on the ACT
                FIFO after the next chunk's sigmoids -- the strict-FIFO ACT
                queue would otherwise head-of-line block on it."""
                ga, gb = gstate.pop((hid_, ch))
                h_prev = None if _h_src(hid_) is None else h_tiles[_h_src(hid_)]
                c_in = c_out = c_real if hid_ < PREV + PRED else c_fake
                if hid_ == PREV + PRED:
                    c_in = c_real  # branch point: fake chain starts from c5
                nch = _nch(hid_)
                cw = BS // nch

                def v3(t, lo):
                    return t[:, lo:lo + 2 * cw].rearrange(
                        "p (s n) -> p s n", s=2, n=cw)

                i3, f3 = v3(ga, 0), v3(ga, 2 * cw)
                o3, sg3 = v3(gb, 0), v3(gb, 2 * cw)

                def cvw(t):
                    return t[:, :].rearrange("p (s c n) -> p s c n",
                                             s=2, c=nch, n=cw)[:, :, ch]

                cvi, cvo = cvw(c_in), cvw(c_out)
                # c = f*c_prev + i*(2*sg - 1) reassociated as
                #     (f*c_prev - i) + (2i)*sg
                # so everything except the last two DVE ops depends only on
                # the EARLIER sigmoid (za): the serial chain after sigmoid(zb)
                # stays two ops long, same as with a native tanh g-gate.
                i2 = tpool.tile([128, 2 * cw], bf16, tag="g2", name="i2")
                vv = tpool.tile([128, 2 * cw], bf16, tag="vv", name="vv")
                nc.vector.tensor_scalar_mul(v3(i2, 0), i3, 2.0)
                if h_prev is None:
                    nc.vector.tensor_tensor(v3(vv, 0), v3(i2, 0), sg3, OP.mult)
                    nc.vector.tensor_tensor(cvo, v3(vv, 0), i3, OP.subtract)
                else:
                    fc = tpool.tile([128, 2 * cw], bf16, tag="fc", name="fc")
                    ig = tpool.tile([128, 2 * cw], bf16, tag="ig", name="ig")
                    nc.vector.tensor_tensor(v3(fc, 0), f3, cvi, OP.mult)
                    nc.vector.tensor_tensor(v3(ig, 0), v3(fc, 0), i3, OP.subtract)
                    nc.vector.tensor_tensor(v3(vv, 0), v3(i2, 0), sg3, OP.mult)
                    nc.vector.tensor_tensor(cvo, v3(ig, 0), v3(vv, 0), OP.add)
                tcn = tpool.tile([128, 2 * cw], bf16, tag="tc", name="tc")
                nc.scalar.activation(v3(tcn, 0), cvo, AF.Tanh)
                hv = h_tiles[hid_][:, :, ch * cw:(ch + 1) * cw]
                nc.vector.tensor_tensor(hv, o3, v3(tcn, 0), OP.mult)

            def emit_dense(q_):
                """pred[:, 4q:4q+4] for both branches in ONE 4-bank PSUM
                tile: partitions {0 real, 32 fake} via tile_position, the 4
                steps of the quad on free quarters, so a single
                sigmoid(+bias) drains it. 3 emissions total (4+4+2 steps).
                (2-step pairs were A/B-tested: consistently ~4us slower
                single-shot -- the extra sigmoids and extra PSUM-chain
                disruptions outweigh the shorter matmul bursts.)"""
                nstep = min(4, PRED - 4 * q_)
                dp = zpool.tile([128, nstep * BS], f32, tag="ta", name="dp",
                                bufs=1)
                for dt_ in range(nstep):
                    for br, j0 in ((0, 0), (1, 32)):
                        cell = (PREV if br == 0 else PREV + PRED) + 4 * q_ + dt_
                        for s in range(2):
                            nc.tensor.matmul(
                                dp[j0:j0 + 1, dt_ * BS:(dt_ + 1) * BS],
                                dw_t[:, s:s + 1],
                                h_tiles[cell][:, s, :],
                                start=(s == 0), stop=(s == 1),
                                tile_position=(0, j0),
                            )
                nc.scalar.activation(
                    dsig[:, 4 * q_ * BS:(4 * q_ + nstep) * BS],
                    dp[0:33, 0:nstep * BS],
                    AF.Sigmoid, bias=db_t[0:33, 0:1])

            def emit_body():
              units = []
              for hid_ in CELL_ORDER:
                  for ch in range(_nch(hid_)):
                      units.append((hid_, ch))

              x_tiles = {}
              pending = None
              for hid_, ch in units:
                if ch == 0:
                    x_t = xpool.tile([FEAT + 1, BS], bf16, tag="x", name="x")
                    nc.sync.dma_start(x_t[:, :], xT[hid_])
                    x_tiles[hid_] = x_t

                if hid_ == PREV + PRED and pending is not None:
                    # branch point: the first fake cell reads BOTH chunks of
                    # h5, so the delayed phase-2 of (5, 1) must land first
                    emit_p2(*pending)
                    pending = None

                emit_p1(hid_, ch, x_tiles[hid_])
                if pending is not None:
                    emit_p2(*pending)
                pending = (hid_, ch)

                t_r = hid_ - PREV  # real cell completes step t_r
                if PREV <= hid_ < PREV + PRED and t_r in (5, 9):
                    # dense quad (4q..4q+3), emitted behind the scan
                    emit_dense((t_r - 5) // 4)

              emit_p2(*pending)
              emit_dense(2)

              nc.sync.dma_start(outT[0], dsig[0:1, :])
              nc.sync.dma_start(outT[1], dsig[32:33, :])

            if loop_r is None:
                emit_body()
            else:
                with tc.For_i(0, loop_r, 1,
                              hint_engines=(mybir.EngineType.PE,)):
                    emit_body()

    nc.compile()
    return nc


_PROGRAMS = {}


def _get_program(loop_r=None):
    if loop_r not in _PROGRAMS:
        _PROGRAMS[loop_r] = _build_program(loop_r)
    return _PROGRAMS[loop_r]


def _prep_inputs(real_input, fake_input, kernel, recurrent_kernel, bias, dense_w,
                 dense_b):
    kernel_p = np.asarray(kernel, np.float32)[:, PERM]
    bias_p = np.asarray(bias, np.float32)[PERM]
    wh_p = np.asarray(recurrent_kernel, np.float32)[:, PERM]
    # double the g-gate columns (permuted banks 6,7): tanh(z) = 2*sig(2z)-1
    kernel_p[:, 6 * 128:] *= 2.0
    bias_p[6 * 128:] *= 2.0
    wh_p[:, 6 * 128:] *= 2.0

    wx_aug = np.concatenate([kernel_p, bias_p[None]], 0).astype(BF16)  # [65,1024]
    # wh_sb[p, s, j] = wh_p[s*128+p, j]  (k-tile layout)
    wh_sb = np.ascontiguousarray(
        wh_p.reshape(2, 128, H4).transpose(1, 0, 2)
    ).astype(BF16)
    dw_sb = np.ascontiguousarray(
        np.asarray(dense_w, np.float32)[:, 0].reshape(2, 128).T
    ).astype(BF16)
    db = np.full((128, 1), float(np.asarray(dense_b).reshape(())), np.float32)

    # x cells: 0..15 real steps, 16..25 fake steps; transposed + ones row
    xcat = np.concatenate(
        [np.asarray(real_input, np.float32), np.asarray(fake_input, np.float32)],
        axis=1,
    )  # [B, 26, 64]
    xT = np.transpose(xcat, (1, 2, 0))  # [26, 64, B]
    xT = np.concatenate([xT, np.ones((NCELL, 1, B), np.float32)], axis=1)
    xT = xT.astype(BF16)  # [26, 65, B]

    in_maps = []
    for c in range(N_CORES):
        in_maps.append({
            "xT": np.ascontiguousarray(xT[:, :, c * BS:(c + 1) * BS]),
            "wx": wx_aug,
            "wh": wh_sb,
            "dw": dw_sb,
            "dbias": db,
        })
    return in_maps


_EXECS = {}


def _get_exec(loop_r=None):
    """Cached shard_map executable over the 8 cores (mirrors
    bass2jax.run_bass_via_pjrt but reusable across calls)."""
    if loop_r in _EXECS:
        return _EXECS[loop_r]

    import jax
    from jax.sharding import Mesh, PartitionSpec, NamedSharding
    from jax.experimental.shard_map import shard_map
    from concourse.bass2jax import (_bass_exec_p, install_neuronx_cc_hook,
                                    partition_id_tensor)

    install_neuronx_cc_hook()
    nc = _get_program(loop_r)

    partition_name = nc.partition_id_tensor.name if nc.partition_id_tensor else None
    in_names, out_names, out_avals, zero_outs = [], [], [], []
    for alloc in nc.m.functions[0].allocations:
        if not isinstance(alloc, mybir.MemoryLocationSet):
            continue
        name = alloc.memorylocations[0].name
        if alloc.kind == "ExternalInput":
            if name != partition_name:
                in_names.append(name)
        elif alloc.kind == "ExternalOutput":
            out_names.append(name)
            shape = tuple(alloc.tensor_shape)
            dtype = mybir.dt.np(alloc.dtype)
            out_avals.append(jax.core.ShapedArray(shape, dtype))
            zero_outs.append(np.zeros(shape, dtype))
    n_params = len(in_names)
    all_in_names = in_names + out_names
    if partition_name is not None:
        all_in_names = all_in_names + [partition_name]

    def _body(*args):
        operands = list(args)
        if partition_name is not None:
            operands.append(partition_id_tensor())
        outs = _bass_exec_p.bind(
            *operands,
            out_avals=tuple(out_avals),
            in_names=tuple(all_in_names),
            out_names=tuple(out_names),
            lowering_input_output_aliases=(),
            sim_require_finite=True,
            sim_require_nnan=True,
            nc=nc,
        )
        return tuple(outs)

    devices = jax.devices()[:N_CORES]
    mesh = Mesh(np.asarray(devices), ("core",))
    n_args = n_params + len(out_names)
    fn = jax.jit(
        shard_map(_body, mesh=mesh,
                  in_specs=(PartitionSpec("core"),) * n_args,
                  out_specs=(PartitionSpec("core"),) * len(out_names),
                  check_rep=False),
        keep_unused=True,
    )
    sharding = NamedSharding(mesh, PartitionSpec("core"))
    _EXECS[loop_r] = dict(fn=fn, in_names=in_names, out_names=out_names,
                          out_avals=out_avals, zero_outs=zero_outs,
                          sharding=sharding)
    return _EXECS[loop_r]


def _concat_args(ex, in_maps):
    args = [
        np.concatenate([np.asarray(m[name]) for m in in_maps], axis=0)
        for name in ex["in_names"]
    ]
    args += [
        np.zeros((N_CORES * z.shape[0], *z.shape[1:]), z.dtype)
        for z in ex["zero_outs"]
    ]
    return args


def _split_out(ex, out_arrs):
    stacked = np.asarray(out_arrs[0], np.float32).reshape(N_CORES, 2, PRED, BS)
    real = stacked[:, 0].transpose(0, 2, 1).reshape(B, PRED, 1)
    fake = stacked[:, 1].transpose(0, 2, 1).reshape(B, PRED, 1)
    return np.ascontiguousarray(real), np.ascontiguousarray(fake)


def run(inputs):
    """Run once; returns (real_pred, fake_pred)."""
    ex = _get_exec()
    in_maps = _prep_inputs(**inputs)
    out_arrs = ex["fn"](*_concat_args(ex, in_maps))
    return _split_out(ex, out_arrs)


def bench(inputs, iters=32):
    """Steady-state timing: device-resident args, async dispatch loop."""
    tn, _ = _bench_exec(None, inputs, iters)
    return tn, tn


def _bench_prep(loop_r, inputs):
    import jax

    ex = _get_exec(loop_r)
    in_maps = _prep_inputs(**inputs)
    args = [jax.device_put(a, ex["sharding"]) for a in _concat_args(ex, in_maps)]
    for a in args:
        a.block_until_ready()
    out = ex["fn"](*args)  # warmup / compile
    jax.block_until_ready(out)
    return ex, args


def bench_hw(inputs, r_hi=128, r_lo=8, samples=10):
    """Per-NEFF-iteration HW time via in-kernel For_i loop: INTERLEAVED
    (r_hi, r_lo) dispatch pairs -- the terminal's clock drifts on the
    scale of minutes, so block-mode (all r_hi then all r_lo) produces
    diffs off by +-30%; per-pair diffs taken seconds apart cancel the
    drift, and the median rejects stragglers. Alternating executables
    also forces a NEFF reload on every dispatch, which lands on both
    variants equally and cancels in the diff."""
    import jax
    import time

    def one(ex, args):
        t0 = time.perf_counter()
        out = ex["fn"](*args)
        jax.block_until_ready(out)
        return time.perf_counter() - t0

    ex_hi, args_hi = _bench_prep(r_hi, inputs)
    ex_lo, args_lo = _bench_prep(r_lo, inputs)
    one(ex_hi, args_hi)
    one(ex_lo, args_lo)  # absorb first NEFF switches
    diffs = []
    for _ in range(samples):
        t_hi = one(ex_hi, args_hi)
        t_lo = one(ex_lo, args_lo)
        diffs.append((t_hi - t_lo) / (r_hi - r_lo))
    diffs.sort()
    return diffs[len(diffs) // 2], diffs[0], diffs[-1]


def kernel(real_input, fake_input, kernel, recurrent_kernel, bias, dense_w,
           dense_b):
    return run(dict(
        real_input=real_input, fake_input=fake_input, kernel=kernel,
        recurrent_kernel=recurrent_kernel, bias=bias, dense_w=dense_w,
        dense_b=dense_b,
    ))
